# revision 15
# baseline (speedup 1.0000x reference)
"""Distributed Bass kernel: fused multi-head attention block on 8 TRN2 NeuronCores.

Problem: x[2,2048,1024] -> QKV proj -> RoPE(q,k) -> softmax(q k^T/8) v -> out proj.

Sharding: tensor-parallel over heads. 16 heads / 8 cores = 2 heads per core.
Each core computes QKV for its 2 heads (full sequence), RoPE, attention, then
an AllToAll converts head-sharding to token-sharding so the output projection
runs against the FULL Wout with no AllReduce. The AllToAll is split per batch:
batch 0's exchange is triggered as soon as batch 0's attention is normalized,
so its ~30us network+barrier latency hides under batch 1's attention compute.
Core j ends with tokens {b0: [256j,256j+256), b1: [2048+256j, ...)}; the host
re-interleaves the 8 slices.

Pipeline structure:
 - phase 1 computes QKV+RoPE for batch 0 only (chunks 0-3).
 - batch 1's QKV chunks are interleaved into batch 0's attention chunks (the
   attention phase is ACT(exp)-bound, so the PE slack absorbs them).
 - normalize uses a PE indicator-matmul broadcast (NOT gpsimd
   partition_broadcast): the gpsimd queue carries only A2A staging DMAs, so
   the collective doorbell is never stuck behind other work.
 - outproj(b0) emission is delayed to after (b1,qc2) so the tile scheduler
   cannot head-of-line-block the PE on the A2A-gated gather.

Compute dtype bf16 (PE 1 cycle/row), f32 PSUM accumulation. Softmax skips the
max-subtraction (scores ~N(0,2), |s|<~12, exp safe in f32) and folds the
denominator into the PV matmul via a ones-column appended to v.
"""

import sys

for _p in ("/opt/trn_rl_repo", "/root/.axon_site/_ro/trn_rl_repo"):
    if _p not in sys.path:
        sys.path.append(_p)

import numpy as np
import ml_dtypes

B, N, HID = 2, 2048, 1024
H, DH = 16, 64
NCORES = 8
HPC = H // NCORES          # heads per core = 2
T = B * N                  # 4096 flattened tokens
TS = T // NCORES           # 512 tokens per core total (256 per batch)
TSB = TS // B              # 256 tokens per core per batch
EPC = HPC * DH             # 128 features per core
CH = 512                   # token chunk for QKV phase
NCH = T // CH              # 8 chunks
KT = 128                   # key tile
QC = 512                   # query chunk in attention

_bf16 = ml_dtypes.bfloat16


def _build_graph():
    import concourse.bass as bass
    import concourse.mybir as mybir
    import concourse.tile as tile
    from concourse import bacc

    f32 = mybir.dt.float32
    bf16 = mybir.dt.bfloat16

    nc = bacc.Bacc("TRN2", target_bir_lowering=False, debug=False, num_devices=NCORES)

    xT_e = nc.declare_dram_parameter("xT", [HID, T], bf16, isOutput=False)
    wqkvT_e = nc.declare_dram_parameter("wqkvT", [HID, 3 * EPC], bf16, isOutput=False)
    woutT_e = nc.declare_dram_parameter("woutT", [HID, HID], bf16, isOutput=False)
    cos2_e = nc.declare_dram_parameter("cos2", [2 * DH, T], bf16, isOutput=False)
    sin2_e = nc.declare_dram_parameter("sin2", [2 * DH, T], bf16, isOutput=False)
    perm_e = nc.declare_dram_parameter("perm", [128, 128], bf16, isOutput=False)
    ident_e = nc.declare_dram_parameter("ident", [128, 128], bf16, isOutput=False)
    out_e = nc.declare_dram_parameter("out", [TS, HID], f32, isOutput=True)

    with tile.TileContext(nc) as tc:
        with (
            tc.tile_pool(name="const", bufs=1) as cpool,
            tc.tile_pool(name="work", bufs=1) as wpool,
            tc.tile_pool(name="stream", bufs=4) as spool,
            tc.tile_pool(name="psum", bufs=2, space="PSUM") as pspool,
            tc.tile_pool(name="dram", bufs=1, space="DRAM") as dpool,
        ):
            # ---- constants / weights ----
            wqkvT = cpool.tile([128, 8 * 3 * EPC], bf16)       # 8 k-tiles side by side
            for kt in range(8):
                nc.gpsimd.dma_start(
                    wqkvT[:, kt * 3 * EPC:(kt + 1) * 3 * EPC],
                    wqkvT_e[kt * 128:(kt + 1) * 128, :],
                )
            perm = cpool.tile([128, 128], bf16)
            nc.gpsimd.dma_start(perm[:, :], perm_e[:, :])
            ident = cpool.tile([128, 128], bf16)
            nc.gpsimd.dma_start(ident[:, :], ident_e[:, :])
            cos2 = cpool.tile([128, T], bf16)   # loaded per-chunk, streamed
            sin2 = cpool.tile([128, T], bf16)
            woutT = cpool.tile([128, 8 * HID], bf16)
            # ones row for the normalize broadcast: ones1^T @ rec (K=1 matmul)
            # replicates partition 0 of rec into 64 output partitions; the
            # multiply by 1.0 is exact so only rec's bf16 rounding enters
            ones1 = cpool.tile([1, DH], bf16)
            nc.vector.memset(ones1[0:1, :], 1.0)
            wtile = cpool.tile([128, 512], bf16)
            nc.vector.memset(wtile[:, :], 0.0)

            # ---- persistent working tensors ----
            q_sb = wpool.tile([128, T], bf16)      # raw q (rope intermediate)
            k_sb = wpool.tile([128, T], bf16)      # becomes roped k
            qzA = wpool.tile([128, T], bf16)       # roped qA rows 0-63, 0 below
            qzB = wpool.tile([128, T], bf16)       # roped qB rows 64-127, 0 above
            vT_sb = wpool.tile([128, T], bf16)     # v transposed [e, t]
            vexA = wpool.tile([128, 32 * 128], bf16)  # head-A v table per slot
            vexB = wpool.tile([128, 32 * 128], bf16)
            ovT = wpool.tile([128, T], bf16)       # attention out ^T
            gTb = [wpool.tile([128, NCORES * TSB], bf16, name=f"gTb{b}")
                   for b in range(B)]              # post-A2A gathered [e, t]

            nc.vector.memset(qzA[DH:128, :], 0.0)
            nc.vector.memset(qzB[0:DH, :], 0.0)
            vexA3 = vexA.rearrange("p (s c) -> p s c", c=128)
            vexB3 = vexB.rearrange("p (s c) -> p s c", c=128)
            nc.vector.memset(vexA3[:, :, DH:128], 0.0)
            nc.vector.memset(vexB3[:, :, DH:128], 0.0)
            nc.vector.memset(vexA3[:, :, DH:DH + 1], 1.0)
            nc.vector.memset(vexB3[:, :, DH:DH + 1], 1.0)

            # ---------------- QKV + RoPE + v-transpose for one chunk ----------------
            def emit_qkv_chunk(c):
                xs = []
                for kt in range(8):
                    xt = spool.tile([128, CH], bf16, tag="xs", bufs=12)
                    nc.sync.dma_start(
                        xt[:, :], xT_e[kt * 128:(kt + 1) * 128, c * CH:(c + 1) * CH]
                    )
                    xs.append(xt)
                sl = slice(c * CH, (c + 1) * CH)
                nc.sync.dma_start(cos2[:, sl], cos2_e[:, sl])
                nc.sync.dma_start(sin2[:, sl], sin2_e[:, sl])
                for which, dest in ((0, q_sb), (1, k_sb), (2, vT_sb)):
                    ps = pspool.tile([128, CH], f32, tag="mm", bufs=2)
                    for kt in range(8):
                        nc.tensor.matmul(
                            ps[:, :],
                            wqkvT[:, kt * 3 * EPC + which * EPC:
                                  kt * 3 * EPC + (which + 1) * EPC],
                            xs[kt][:, :],
                            start=(kt == 0),
                            stop=(kt == 7),
                        )
                    nc.vector.tensor_copy(dest[:, c * CH:(c + 1) * CH], ps[:, :])

                # RoPE on q and k: t = P@x * sin2 ; rot = x*cos2 + t
                for srd, dests in ((q_sb, ((qzA, 0, DH), (qzB, DH, 128))),
                                   (k_sb, ((k_sb, 0, 128),))):
                    pps = pspool.tile([128, CH], f32, tag="mm", bufs=2)
                    nc.tensor.matmul(
                        pps[:, :], perm[:, :], srd[:, sl],
                        start=True, stop=True,
                    )
                    tmp = spool.tile([128, CH], bf16, tag="ropetmp", bufs=2)
                    nc.vector.tensor_mul(tmp[:, :], pps[:, :], sin2[:, sl])
                    nc.vector.tensor_mul(srd[:, sl], srd[:, sl], cos2[:, sl])
                    for dst, p0, p1 in dests:
                        nc.vector.tensor_add(
                            dst[p0:p1, sl], srd[p0:p1, sl], tmp[p0:p1, :]
                        )

                # transpose v chunk into per-head 128-wide v tables
                for tt in range(CH // 128):
                    slot = c * (CH // 128) + tt
                    tp = pspool.tile([128, 128], bf16, tag="sc", bufs=2)
                    nc.tensor.transpose(
                        tp[:, :],
                        vT_sb[:, c * CH + tt * 128:c * CH + (tt + 1) * 128],
                        ident[:, :],
                    )
                    nc.vector.tensor_copy(vexA3[:, slot, 0:DH], tp[:, 0:DH])
                    nc.vector.tensor_copy(vexB3[:, slot, 0:DH], tp[:, DH:2 * DH])

            # ================= attention machinery =================
            NKT = N // KT                      # 16 key tiles per chunk
            vtabs = (vexA, vexB)

            def emit_pv_pair(st, pair):
                (b, qc, opsAB, expT) = st
                for h in range(HPC):
                    for kt in (2 * pair, 2 * pair + 1):
                        slot = b * (N // 128) + kt
                        nc.tensor.matmul(
                            opsAB[h][:, :],
                            vtabs[h][:, slot * 128:(slot + 1) * 128],
                            expT[:, kt * 1024 + h * QC:kt * 1024 + (h + 1) * QC],
                            start=(kt == 0),
                            stop=(kt == NKT - 1),
                        )

            def emit_normalize(st):
                (b, qc, opsAB, expT) = st
                q0 = b * N + qc * QC
                for h in range(HPC):
                    hr = h * DH
                    den = spool.tile([1, QC], f32, tag="den", bufs=2)
                    nc.vector.tensor_copy(den[0:1, :], opsAB[h][DH:DH + 1, :])
                    rec = spool.tile([1, QC], f32, tag="rec", bufs=2)
                    nc.vector.reciprocal_approx_fast(rec[0:1, :], den[0:1, :])
                    recb = spool.tile([1, QC], bf16, tag="recb", bufs=2)
                    nc.vector.tensor_copy(recb[0:1, :], rec[0:1, :])
                    bcs = pspool.tile([64, QC], f32, tag="sc", bufs=2)
                    nc.tensor.matmul(bcs[:, :], ones1[0:1, :], recb[0:1, :],
                                     start=True, stop=True)
                    # DVE may read only one PSUM operand; bounce bcs to SBUF
                    bcss = spool.tile([64, QC], f32, tag="bcss", bufs=2)
                    nc.vector.tensor_copy(bcss[:, :], bcs[:, :])
                    nc.vector.tensor_mul(
                        ovT[hr:hr + DH, q0:q0 + QC],
                        opsAB[h][0:DH, :], bcss[0:DH, :]
                    )

            def emit_stage(b, qc, a2a_in):
                for half in range(2):
                    j = 2 * qc + half
                    t0 = b * N + qc * QC + half * TSB
                    nc.gpsimd.dma_start(
                        a2a_in[j * 128:(j + 1) * 128, :],
                        ovT[:, t0:t0 + TSB],
                    )

            def emit_outproj(b):
                for m2 in range(TSB // 128):
                    for nn2 in range(HID // 512):
                        odps = pspool.tile([128, 512], f32, tag="mm", bufs=2)
                        for et in range(8):
                            nc.tensor.matmul(
                                odps[:, :],
                                gTb[b][:, et * TSB + m2 * 128:
                                       et * TSB + (m2 + 1) * 128],
                                woutT[:, et * HID + nn2 * 512:
                                      et * HID + (nn2 + 1) * 512],
                                start=(et == 0),
                                stop=(et == 7),
                            )
                        osb = spool.tile([128, 512], f32, tag="osb", bufs=2)
                        nc.vector.tensor_copy(osb[:, :], odps[:, :])
                        # sync queue: keeps gpsimd free for staging/doorbells
                        nc.sync.dma_start(
                            out_e[b * TSB + m2 * 128:b * TSB + (m2 + 1) * 128,
                                  nn2 * 512:(nn2 + 1) * 512],
                            osb[:, :],
                        )

            a2a_in = [dpool.tile([NCORES * 128, TSB], bf16, name=f"a2aI{b}")
                      for b in range(B)]
            a2a_out = [dpool.tile([NCORES * 128, TSB], bf16, name=f"a2aO{b}")
                       for b in range(B)]

            def emit_a2a(b):
                nc.gpsimd.collective_compute(
                    "AllToAll",
                    mybir.AluOpType.bypass,
                    ins=[a2a_in[b].opt()],
                    outs=[a2a_out[b].opt()],
                    replica_groups=[list(range(NCORES))],
                )

            def emit_gather(b):
                # one strided DMA: a2a_out [8(e) x 128(p), 256(t)] ->
                # gTb [128(p), 8(e) x 256(t)]; blocks on the A2A-done
                # semaphore, so it lives on the otherwise-idle sync queue
                src = a2a_out[b].rearrange("(e p) t -> p e t", p=128)
                dst = gTb[b].rearrange("p (e t) -> p e t", e=NCORES)
                nc.sync.dma_start(dst[:, :, :], src[:, :, :])

            # ================= emission =================
            # PE warm-up under the initial DMA wait: keeps HAM busy so the
            # clock-gate releases to 2.4 GHz before the first real matmul
            for w in range(12):
                wps = pspool.tile([128, 512], f32, tag="mm", bufs=2)
                nc.tensor.matmul(wps[:, :], wtile[:, 0:128], wtile[:, :],
                                 start=True, stop=True)
            # phase 1: batch-0 QKV only
            for c in range(4):
                emit_qkv_chunk(c)

            qzs = (qzA, qzB)
            pending = None
            for b in range(B):
                for qc in range(N // QC):
                    q0 = b * N + qc * QC
                    expT = spool.tile([128, NKT * 2 * QC], bf16, name="expT",
                                      tag="expT", bufs=2)
                    for pair in range(NKT // 2):
                        for half in range(2):
                            kt = 2 * pair + half
                            k0 = b * N + kt * KT
                            sps = pspool.tile([128, 2 * QC], f32, tag="sc",
                                              bufs=2)
                            for h in range(HPC):
                                nc.tensor.matmul(
                                    sps[:, h * QC:(h + 1) * QC],
                                    k_sb[:, k0:k0 + KT],
                                    qzs[h][:, q0:q0 + QC],
                                    start=True, stop=True,
                                )
                            nc.scalar.activation(
                                expT[:, kt * 1024:(kt + 1) * 1024],
                                sps[:, :],
                                mybir.ActivationFunctionType.Exp,
                                scale=DH ** -0.5,
                            )
                        if pending is not None:
                            emit_pv_pair(pending, pair)
                            if pair == NKT // 2 - 1:
                                emit_normalize(pending)
                                pb, pqc = pending[0], pending[1]
                                emit_stage(pb, pqc, a2a_in[pb])
                                if pqc == N // QC - 1:
                                    emit_a2a(pb)
                    if b == 0:
                        # fuse batch-1 QKV into batch-0's (ACT-bound)
                        # attention chunks
                        emit_qkv_chunk(4 + qc)
                    if b == 0 and qc == 1:
                        # woutT load here: HBM is otherwise idle mid-attention
                        # (during phase 1 it starved the xs stream)
                        for kt in range(8):
                            nc.gpsimd.dma_start(
                                woutT[:, kt * HID:(kt + 1) * HID],
                                woutT_e[kt * 128:(kt + 1) * 128, :],
                            )
                    if b == 1 and qc == 3:
                        # A2A#0 finished during qc1/qc2; emitting the gather +
                        # outproj here (not earlier) keeps the tile scheduler
                        # from head-of-line-blocking the PE on the A2A
                        emit_gather(0)
                        emit_outproj(0)
                    opsAB = (pspool.tile([128, QC], f32, name="opsA",
                                         tag="aux", bufs=2),
                             pspool.tile([128, QC], f32, name="opsB",
                                         tag="aux", bufs=2))
                    pending = (b, qc, opsAB, expT)
            for pair in range(NKT // 2):
                emit_pv_pair(pending, pair)
            emit_normalize(pending)
            emit_stage(B - 1, N // QC - 1, a2a_in[B - 1])
            emit_a2a(B - 1)
            emit_gather(B - 1)
            emit_outproj(1)

    nc.finalize()
    return nc


def _host_inputs(x, rope, Wqkv, Wout):
    """Build the 8 per-core input maps with host-side layout prep."""
    xf = np.ascontiguousarray(x.reshape(T, HID).T).astype(_bf16)        # [1024, 4096]
    woutT = np.ascontiguousarray(Wout.T).astype(_bf16)                  # [1024, 1024]

    rf = rope.reshape(T, DH)                                            # [4096, 64]
    cosE = np.repeat(rf[:, 0::2], 2, axis=1).T                          # [64, 4096]
    sinE = np.repeat(rf[:, 1::2], 2, axis=1).T
    sgn = np.where(np.arange(DH) % 2 == 0, -1.0, 1.0)[:, None]
    sinS = (sinE * sgn)
    cos2 = np.ascontiguousarray(np.concatenate([cosE, cosE], 0)).astype(_bf16)
    sin2 = np.ascontiguousarray(np.concatenate([sinS, sinS], 0)).astype(_bf16)

    pm = np.zeros((128, 128), np.float32)
    for d in range(128):
        pm[d ^ 1, d] = 1.0       # partner[d] = q[d^1]; lhsT = S (symmetric)
    perm = pm.astype(_bf16)
    ident = np.eye(128, dtype=np.float32).astype(_bf16)

    w3 = Wqkv.reshape(3, H, DH, HID)
    in_maps = []
    for c in range(NCORES):
        blocks = []
        for which in range(3):
            for hl in range(HPC):
                blocks.append(w3[which, 2 * c + hl])                    # [64, 1024]
        wq = np.concatenate(blocks, 0)                                  # [384, 1024]
        wqkvT = np.ascontiguousarray(wq.T).astype(_bf16)                # [1024, 384]
        in_maps.append({
            "xT": xf, "wqkvT": wqkvT, "woutT": woutT,
            "cos2": cos2, "sin2": sin2, "perm": perm, "ident": ident,
        })
    return in_maps


_CACHE = {}


def kernel(x, rope, Wqkv, Wout):
    from concourse.bass_utils import run_bass_kernel_spmd

    if "nc" not in _CACHE:
        _CACHE["nc"] = _build_graph()
    nc = _CACHE["nc"]
    in_maps = _host_inputs(np.asarray(x, np.float32), np.asarray(rope, np.float32),
                           np.asarray(Wqkv, np.float32), np.asarray(Wout, np.float32))
    res = run_bass_kernel_spmd(nc, in_maps, core_ids=list(range(NCORES)))
    parts = [np.asarray(res.results[i]["out"], np.float32) for i in range(NCORES)]
    full = np.empty((T, HID), np.float32)
    for j in range(NCORES):
        full[j * TSB:(j + 1) * TSB] = parts[j][:TSB]
        full[N + j * TSB:N + (j + 1) * TSB] = parts[j][TSB:]
    return full.reshape(B, N, HID)


# revision 18
# speedup vs baseline: 1.0329x; 1.0329x over previous
"""Distributed Bass kernel: fused multi-head attention block on 8 TRN2 NeuronCores.

Problem: x[2,2048,1024] -> QKV proj -> RoPE(q,k) -> softmax(q k^T/8) v -> out proj.

Sharding: tensor-parallel over heads. 16 heads / 8 cores = 2 heads per core.
Each core computes QKV for its 2 heads (full sequence), RoPE, attention, then
an AllToAll converts head-sharding to token-sharding so the output projection
runs against the FULL Wout with no AllReduce. The AllToAll is split per batch:
batch 0's exchange is triggered as soon as batch 0's attention is normalized,
so its ~30us network+barrier latency hides under batch 1's attention compute.
Core j ends with tokens {b0: [256j,256j+256), b1: [2048+256j, ...)}; the host
re-interleaves the 8 slices.

Pipeline structure:
 - phase 1 computes QKV+RoPE for batch 0 only (chunks 0-3).
 - batch 1's QKV chunks are interleaved into batch 0's attention chunks (the
   attention phase is ACT(exp)-bound, so the PE slack absorbs them).
 - normalize uses a PE indicator-matmul broadcast (NOT gpsimd
   partition_broadcast): the gpsimd queue carries only A2A staging DMAs, so
   the collective doorbell is never stuck behind other work.
 - outproj(b0) emission is delayed to after (b1,qc2) so the tile scheduler
   cannot head-of-line-block the PE on the A2A-gated gather.

Compute dtype bf16 (PE 1 cycle/row), f32 PSUM accumulation. Softmax skips the
max-subtraction (scores ~N(0,2), |s|<~12, exp safe in f32) and folds the
denominator into the PV matmul via a ones-column appended to v.
"""

import sys

for _p in ("/opt/trn_rl_repo", "/root/.axon_site/_ro/trn_rl_repo"):
    if _p not in sys.path:
        sys.path.append(_p)

import numpy as np
import ml_dtypes

B, N, HID = 2, 2048, 1024
H, DH = 16, 64
NCORES = 8
HPC = H // NCORES          # heads per core = 2
T = B * N                  # 4096 flattened tokens
TS = T // NCORES           # 512 tokens per core total (256 per batch)
TSB = TS // B              # 256 tokens per core per batch
EPC = HPC * DH             # 128 features per core
CH = 512                   # token chunk for QKV phase
NCH = T // CH              # 8 chunks
KT = 128                   # key tile
QC = 512                   # query chunk in attention

_bf16 = ml_dtypes.bfloat16


def _build_graph():
    import concourse.bass as bass
    import concourse.mybir as mybir
    import concourse.tile as tile
    from concourse import bacc

    f32 = mybir.dt.float32
    bf16 = mybir.dt.bfloat16

    nc = bacc.Bacc("TRN2", target_bir_lowering=False, debug=False, num_devices=NCORES)

    xT_e = nc.declare_dram_parameter("xT", [HID, T], bf16, isOutput=False)
    wqkvT_e = nc.declare_dram_parameter("wqkvT", [HID, 3 * EPC], bf16, isOutput=False)
    woutT_e = nc.declare_dram_parameter("woutT", [HID, HID], bf16, isOutput=False)
    cos2_e = nc.declare_dram_parameter("cos2", [2 * DH, T], bf16, isOutput=False)
    sin2_e = nc.declare_dram_parameter("sin2", [2 * DH, T], bf16, isOutput=False)
    perm_e = nc.declare_dram_parameter("perm", [128, 128], bf16, isOutput=False)
    ident_e = nc.declare_dram_parameter("ident", [128, 128], bf16, isOutput=False)
    out_e = nc.declare_dram_parameter("out", [TS, HID], f32, isOutput=True)

    with tile.TileContext(nc) as tc:
        with (
            tc.tile_pool(name="const", bufs=1) as cpool,
            tc.tile_pool(name="work", bufs=1) as wpool,
            tc.tile_pool(name="stream", bufs=4) as spool,
            tc.tile_pool(name="psum", bufs=2, space="PSUM") as pspool,
            tc.tile_pool(name="dram", bufs=1, space="DRAM") as dpool,
        ):
            # ---- constants / weights ----
            wqkvT = cpool.tile([128, 8 * 3 * EPC], bf16)       # 8 k-tiles side by side
            for kt in range(8):
                nc.gpsimd.dma_start(
                    wqkvT[:, kt * 3 * EPC:(kt + 1) * 3 * EPC],
                    wqkvT_e[kt * 128:(kt + 1) * 128, :],
                )
            perm = cpool.tile([128, 128], bf16)
            nc.gpsimd.dma_start(perm[:, :], perm_e[:, :])
            ident = cpool.tile([128, 128], bf16)
            nc.gpsimd.dma_start(ident[:, :], ident_e[:, :])
            cos2 = cpool.tile([128, T], bf16)   # loaded per-chunk, streamed
            sin2 = cpool.tile([128, T], bf16)
            woutT = cpool.tile([128, 8 * HID], bf16)
            # ones row for the normalize broadcast: ones1^T @ rec (K=1 matmul)
            # replicates partition 0 of rec into 64 output partitions; the
            # multiply by 1.0 is exact so only rec's bf16 rounding enters
            ones1 = cpool.tile([1, DH], bf16)
            nc.vector.memset(ones1[0:1, :], 1.0)

            # ---- persistent working tensors ----
            q_sb = wpool.tile([128, T], bf16)      # raw q (rope intermediate)
            k_sb = wpool.tile([128, T], bf16)      # becomes roped k
            qzA = wpool.tile([128, T], bf16)       # roped qA rows 0-63, 0 below
            qzB = wpool.tile([128, T], bf16)       # roped qB rows 64-127, 0 above
            vT_sb = wpool.tile([128, T], bf16)     # v transposed [e, t]
            vexA = wpool.tile([128, 32 * 128], bf16)  # head-A v table per slot
            vexB = wpool.tile([128, 32 * 128], bf16)
            ovT = wpool.tile([128, T], bf16)       # attention out ^T
            PW = (TSB, TSB // 2, TSB // 2)     # per-core tokens per A2A piece
            PR = (0, TSB, TSB + TSB // 2)      # out_e row base per piece
            gTb = [wpool.tile([128, NCORES * PW[p]], bf16, name=f"gTb{p}")
                   for p in range(3)]              # post-A2A gathered [e, t]

            nc.vector.memset(qzA[DH:128, :], 0.0)
            nc.vector.memset(qzB[0:DH, :], 0.0)
            vexA3 = vexA.rearrange("p (s c) -> p s c", c=128)
            vexB3 = vexB.rearrange("p (s c) -> p s c", c=128)
            nc.vector.memset(vexA3[:, :, DH:128], 0.0)
            nc.vector.memset(vexB3[:, :, DH:128], 0.0)
            nc.vector.memset(vexA3[:, :, DH:DH + 1], 1.0)
            nc.vector.memset(vexB3[:, :, DH:DH + 1], 1.0)

            # ---------------- QKV + RoPE + v-transpose for one chunk ----------------
            def emit_qkv_chunk(c):
                xs = []
                for kt in range(8):
                    xt = spool.tile([128, CH], bf16, tag="xs", bufs=12)
                    nc.gpsimd.dma_start(
                        xt[:, :], xT_e[kt * 128:(kt + 1) * 128, c * CH:(c + 1) * CH]
                    )
                    xs.append(xt)
                sl = slice(c * CH, (c + 1) * CH)
                nc.gpsimd.dma_start(cos2[:, sl], cos2_e[:, sl])
                nc.gpsimd.dma_start(sin2[:, sl], sin2_e[:, sl])
                for which, dest in ((0, q_sb), (1, k_sb), (2, vT_sb)):
                    ps = pspool.tile([128, CH], f32, tag="mm", bufs=2)
                    for kt in range(8):
                        nc.tensor.matmul(
                            ps[:, :],
                            wqkvT[:, kt * 3 * EPC + which * EPC:
                                  kt * 3 * EPC + (which + 1) * EPC],
                            xs[kt][:, :],
                            start=(kt == 0),
                            stop=(kt == 7),
                        )
                    nc.vector.tensor_copy(dest[:, c * CH:(c + 1) * CH], ps[:, :])

                # RoPE on q and k: t = P@x * sin2 ; rot = x*cos2 + t
                for srd, dests in ((q_sb, ((qzA, 0, DH), (qzB, DH, 128))),
                                   (k_sb, ((k_sb, 0, 128),))):
                    pps = pspool.tile([128, CH], f32, tag="mm", bufs=2)
                    nc.tensor.matmul(
                        pps[:, :], perm[:, :], srd[:, sl],
                        start=True, stop=True,
                    )
                    tmp = spool.tile([128, CH], bf16, tag="ropetmp", bufs=2)
                    nc.vector.tensor_mul(tmp[:, :], pps[:, :], sin2[:, sl])
                    nc.vector.tensor_mul(srd[:, sl], srd[:, sl], cos2[:, sl])
                    for dst, p0, p1 in dests:
                        nc.vector.tensor_add(
                            dst[p0:p1, sl], srd[p0:p1, sl], tmp[p0:p1, :]
                        )

                # transpose v chunk into per-head 128-wide v tables
                for tt in range(CH // 128):
                    slot = c * (CH // 128) + tt
                    tp = pspool.tile([128, 128], bf16, tag="sc", bufs=2)
                    nc.tensor.transpose(
                        tp[:, :],
                        vT_sb[:, c * CH + tt * 128:c * CH + (tt + 1) * 128],
                        ident[:, :],
                    )
                    nc.vector.tensor_copy(vexA3[:, slot, 0:DH], tp[:, 0:DH])
                    nc.vector.tensor_copy(vexB3[:, slot, 0:DH], tp[:, DH:2 * DH])

            # ================= attention machinery =================
            NKT = N // KT                      # 16 key tiles per chunk
            vtabs = (vexA, vexB)

            def emit_pv_pair(st, pair):
                (b, qc, opsAB, expT) = st
                for h in range(HPC):
                    for kt in (2 * pair, 2 * pair + 1):
                        slot = b * (N // 128) + kt
                        nc.tensor.matmul(
                            opsAB[h][:, :],
                            vtabs[h][:, slot * 128:(slot + 1) * 128],
                            expT[:, kt * 1024 + h * QC:kt * 1024 + (h + 1) * QC],
                            start=(kt == 0),
                            stop=(kt == NKT - 1),
                        )

            def emit_normalize(st):
                (b, qc, opsAB, expT) = st
                q0 = b * N + qc * QC
                for h in range(HPC):
                    hr = h * DH
                    den = spool.tile([1, QC], f32, tag="den", bufs=2)
                    nc.vector.tensor_copy(den[0:1, :], opsAB[h][DH:DH + 1, :])
                    rec = spool.tile([1, QC], f32, tag="rec", bufs=2)
                    nc.vector.reciprocal_approx_fast(rec[0:1, :], den[0:1, :])
                    recb = spool.tile([1, QC], bf16, tag="recb", bufs=2)
                    nc.vector.tensor_copy(recb[0:1, :], rec[0:1, :])
                    bcs = pspool.tile([64, QC], f32, tag="sc", bufs=2)
                    nc.tensor.matmul(bcs[:, :], ones1[0:1, :], recb[0:1, :],
                                     start=True, stop=True)
                    # DVE may read only one PSUM operand; bounce bcs to SBUF
                    bcss = spool.tile([64, QC], f32, tag="bcss", bufs=2)
                    nc.vector.tensor_copy(bcss[:, :], bcs[:, :])
                    nc.vector.tensor_mul(
                        ovT[hr:hr + DH, q0:q0 + QC],
                        opsAB[h][0:DH, :], bcss[0:DH, :]
                    )

            def emit_stage(b, qc):
                # ship this finished chunk's blocks into its piece's A2A input
                if b == 0:
                    p, W, nb, jb = 0, TSB, 2, 2 * qc
                else:
                    p, W, nb, jb = 1 + qc // 2, TSB // 2, 4, 4 * (qc % 2)
                for i in range(nb):
                    t0 = b * N + qc * QC + i * W
                    nc.gpsimd.dma_start(
                        a2a_in[p][(jb + i) * 128:(jb + i + 1) * 128, :],
                        ovT[:, t0:t0 + W],
                    )

            def emit_outproj(p):
                W = PW[p]
                for m2 in range(W // 128):
                    for nn2 in range(HID // 512):
                        odps = pspool.tile([128, 512], f32, tag="mm", bufs=2)
                        for et in range(8):
                            nc.tensor.matmul(
                                odps[:, :],
                                gTb[p][:, et * W + m2 * 128:
                                       et * W + (m2 + 1) * 128],
                                woutT[:, et * HID + nn2 * 512:
                                      et * HID + (nn2 + 1) * 512],
                                start=(et == 0),
                                stop=(et == 7),
                            )
                        osb = spool.tile([128, 512], f32, tag="osb", bufs=2)
                        nc.vector.tensor_copy(osb[:, :], odps[:, :])
                        # sync queue: keeps gpsimd free for staging/doorbells
                        nc.sync.dma_start(
                            out_e[PR[p] + m2 * 128:PR[p] + (m2 + 1) * 128,
                                  nn2 * 512:(nn2 + 1) * 512],
                            osb[:, :],
                        )

            a2a_in = [dpool.tile([NCORES * 128, PW[p]], bf16, name=f"a2aI{p}")
                      for p in range(3)]
            a2a_out = [dpool.tile([NCORES * 128, PW[p]], bf16, name=f"a2aO{p}")
                       for p in range(3)]

            def emit_a2a(p):
                nc.gpsimd.collective_compute(
                    "AllToAll",
                    mybir.AluOpType.bypass,
                    ins=[a2a_in[p].opt()],
                    outs=[a2a_out[p].opt()],
                    replica_groups=[list(range(NCORES))],
                )

            def emit_gather(p):
                # one strided DMA: a2a_out [8(e) x 128(p), W(t)] ->
                # gTb [128(p), 8(e) x W(t)]; blocks on the A2A-done
                # semaphore, so it lives on the otherwise-idle sync queue
                src = a2a_out[p].rearrange("(e p) t -> p e t", p=128)
                dst = gTb[p].rearrange("p (e t) -> p e t", e=NCORES)
                nc.sync.dma_start(dst[:, :, :], src[:, :, :])

            # ================= emission =================
            # phase 1: batch-0 QKV only
            for c in range(4):
                emit_qkv_chunk(c)

            for kt in range(8):
                nc.gpsimd.dma_start(
                    woutT[:, kt * HID:(kt + 1) * HID],
                    woutT_e[kt * 128:(kt + 1) * 128, :],
                )

            qzs = (qzA, qzB)
            pending = None
            for b in range(B):
                for qc in range(N // QC):
                    q0 = b * N + qc * QC
                    expT = spool.tile([128, NKT * 2 * QC], bf16, name="expT",
                                      tag="expT", bufs=2)
                    for pair in range(NKT // 2):
                        for half in range(2):
                            kt = 2 * pair + half
                            k0 = b * N + kt * KT
                            sps = pspool.tile([128, 2 * QC], f32, tag="sc",
                                              bufs=2)
                            for h in range(HPC):
                                nc.tensor.matmul(
                                    sps[:, h * QC:(h + 1) * QC],
                                    k_sb[:, k0:k0 + KT],
                                    qzs[h][:, q0:q0 + QC],
                                    start=True, stop=True,
                                )
                            nc.scalar.activation(
                                expT[:, kt * 1024:(kt + 1) * 1024],
                                sps[:, :],
                                mybir.ActivationFunctionType.Exp,
                                scale=DH ** -0.5,
                            )
                        if pending is not None:
                            emit_pv_pair(pending, pair)
                            if pair == NKT // 2 - 1:
                                emit_normalize(pending)
                                pb, pqc = pending[0], pending[1]
                                emit_stage(pb, pqc)
                                if (pb, pqc) == (0, 3):
                                    emit_a2a(0)
                                elif (pb, pqc) == (1, 1):
                                    emit_a2a(1)
                    if b == 0:
                        # fuse batch-1 QKV into batch-0's (ACT-bound)
                        # attention chunks
                        emit_qkv_chunk(4 + qc)
                    if b == 1 and qc == 3:
                        # A2A#0 finished during qc1/qc2; emitting the gather +
                        # outproj here (not earlier) keeps the tile scheduler
                        # from head-of-line-blocking the PE on the A2A
                        emit_gather(0)
                        emit_outproj(0)
                    opsAB = (pspool.tile([128, QC], f32, name="opsA",
                                         tag="aux", bufs=2),
                             pspool.tile([128, QC], f32, name="opsB",
                                         tag="aux", bufs=2))
                    pending = (b, qc, opsAB, expT)
            for pair in range(NKT // 2):
                emit_pv_pair(pending, pair)
            emit_normalize(pending)
            emit_stage(1, 3)
            emit_a2a(2)
            emit_gather(1)
            emit_outproj(1)
            emit_gather(2)
            emit_outproj(2)

    nc.finalize()
    return nc


def _host_inputs(x, rope, Wqkv, Wout):
    """Build the 8 per-core input maps with host-side layout prep."""
    xf = np.ascontiguousarray(x.reshape(T, HID).T).astype(_bf16)        # [1024, 4096]
    woutT = np.ascontiguousarray(Wout.T).astype(_bf16)                  # [1024, 1024]

    rf = rope.reshape(T, DH)                                            # [4096, 64]
    cosE = np.repeat(rf[:, 0::2], 2, axis=1).T                          # [64, 4096]
    sinE = np.repeat(rf[:, 1::2], 2, axis=1).T
    sgn = np.where(np.arange(DH) % 2 == 0, -1.0, 1.0)[:, None]
    sinS = (sinE * sgn)
    cos2 = np.ascontiguousarray(np.concatenate([cosE, cosE], 0)).astype(_bf16)
    sin2 = np.ascontiguousarray(np.concatenate([sinS, sinS], 0)).astype(_bf16)

    pm = np.zeros((128, 128), np.float32)
    for d in range(128):
        pm[d ^ 1, d] = 1.0       # partner[d] = q[d^1]; lhsT = S (symmetric)
    perm = pm.astype(_bf16)
    ident = np.eye(128, dtype=np.float32).astype(_bf16)

    w3 = Wqkv.reshape(3, H, DH, HID)
    in_maps = []
    for c in range(NCORES):
        blocks = []
        for which in range(3):
            for hl in range(HPC):
                blocks.append(w3[which, 2 * c + hl])                    # [64, 1024]
        wq = np.concatenate(blocks, 0)                                  # [384, 1024]
        wqkvT = np.ascontiguousarray(wq.T).astype(_bf16)                # [1024, 384]
        in_maps.append({
            "xT": xf, "wqkvT": wqkvT, "woutT": woutT,
            "cos2": cos2, "sin2": sin2, "perm": perm, "ident": ident,
        })
    return in_maps


_CACHE = {}


def kernel(x, rope, Wqkv, Wout):
    from concourse.bass_utils import run_bass_kernel_spmd

    if "nc" not in _CACHE:
        _CACHE["nc"] = _build_graph()
    nc = _CACHE["nc"]
    in_maps = _host_inputs(np.asarray(x, np.float32), np.asarray(rope, np.float32),
                           np.asarray(Wqkv, np.float32), np.asarray(Wout, np.float32))
    res = run_bass_kernel_spmd(nc, in_maps, core_ids=list(range(NCORES)))
    parts = [np.asarray(res.results[i]["out"], np.float32) for i in range(NCORES)]
    full = np.empty((T, HID), np.float32)
    HW = TSB // 2
    for j in range(NCORES):
        full[j * TSB:(j + 1) * TSB] = parts[j][:TSB]
        full[N + j * HW:N + (j + 1) * HW] = parts[j][TSB:TSB + HW]
        full[N + N // 2 + j * HW:N + N // 2 + (j + 1) * HW] = parts[j][TSB + HW:]
    return full.reshape(B, N, HID)


# revision 19
# speedup vs baseline: 1.0562x; 1.0225x over previous
"""Distributed Bass kernel: fused multi-head attention block on 8 TRN2 NeuronCores.

Problem: x[2,2048,1024] -> QKV proj -> RoPE(q,k) -> softmax(q k^T/8) v -> out proj.

Sharding: tensor-parallel over heads. 16 heads / 8 cores = 2 heads per core.
Each core computes QKV for its 2 heads (full sequence), RoPE, attention, then
an AllToAll converts head-sharding to token-sharding so the output projection
runs against the FULL Wout with no AllReduce. The AllToAll is split per batch:
batch 0's exchange is triggered as soon as batch 0's attention is normalized,
so its ~30us network+barrier latency hides under batch 1's attention compute.
Core j ends with tokens {b0: [256j,256j+256), b1: [2048+256j, ...)}; the host
re-interleaves the 8 slices.

Pipeline structure:
 - phase 1 computes QKV+RoPE for batch 0 only (chunks 0-3).
 - batch 1's QKV chunks are interleaved into batch 0's attention chunks (the
   attention phase is ACT(exp)-bound, so the PE slack absorbs them).
 - normalize uses a PE indicator-matmul broadcast (NOT gpsimd
   partition_broadcast): the gpsimd queue carries only A2A staging DMAs, so
   the collective doorbell is never stuck behind other work.
 - outproj(b0) emission is delayed to after (b1,qc2) so the tile scheduler
   cannot head-of-line-block the PE on the A2A-gated gather.

Compute dtype bf16 (PE 1 cycle/row), f32 PSUM accumulation. Softmax skips the
max-subtraction (scores ~N(0,2), |s|<~12, exp safe in f32) and folds the
denominator into the PV matmul via a ones-column appended to v.
"""

import sys

for _p in ("/opt/trn_rl_repo", "/root/.axon_site/_ro/trn_rl_repo"):
    if _p not in sys.path:
        sys.path.append(_p)

import numpy as np
import ml_dtypes

B, N, HID = 2, 2048, 1024
H, DH = 16, 64
NCORES = 8
HPC = H // NCORES          # heads per core = 2
T = B * N                  # 4096 flattened tokens
TS = T // NCORES           # 512 tokens per core total (256 per batch)
TSB = TS // B              # 256 tokens per core per batch
EPC = HPC * DH             # 128 features per core
CH = 512                   # token chunk for QKV phase
NCH = T // CH              # 8 chunks
KT = 128                   # key tile
QC = 512                   # query chunk in attention

_bf16 = ml_dtypes.bfloat16


def _build_graph():
    import concourse.bass as bass
    import concourse.mybir as mybir
    import concourse.tile as tile
    from concourse import bacc

    f32 = mybir.dt.float32
    bf16 = mybir.dt.bfloat16

    nc = bacc.Bacc("TRN2", target_bir_lowering=False, debug=False, num_devices=NCORES)

    xT_e = nc.declare_dram_parameter("xT", [HID, T], bf16, isOutput=False)
    wqkvT_e = nc.declare_dram_parameter("wqkvT", [HID, 3 * EPC], bf16, isOutput=False)
    woutT_e = nc.declare_dram_parameter("woutT", [HID, HID], bf16, isOutput=False)
    cos2_e = nc.declare_dram_parameter("cos2", [2 * DH, T], bf16, isOutput=False)
    sin2_e = nc.declare_dram_parameter("sin2", [2 * DH, T], bf16, isOutput=False)
    perm_e = nc.declare_dram_parameter("perm", [128, 128], bf16, isOutput=False)
    ident_e = nc.declare_dram_parameter("ident", [128, 128], bf16, isOutput=False)
    out_e = nc.declare_dram_parameter("out", [TS, HID], f32, isOutput=True)

    with tile.TileContext(nc) as tc:
        with (
            tc.tile_pool(name="const", bufs=1) as cpool,
            tc.tile_pool(name="work", bufs=1) as wpool,
            tc.tile_pool(name="stream", bufs=4) as spool,
            tc.tile_pool(name="psum", bufs=2, space="PSUM") as pspool,
            tc.tile_pool(name="dram", bufs=1, space="DRAM") as dpool,
        ):
            # ---- constants / weights ----
            wqkvT = cpool.tile([128, 8 * 3 * EPC], bf16)       # 8 k-tiles side by side
            for kt in range(8):
                nc.gpsimd.dma_start(
                    wqkvT[:, kt * 3 * EPC:(kt + 1) * 3 * EPC],
                    wqkvT_e[kt * 128:(kt + 1) * 128, :],
                )
            perm = cpool.tile([128, 128], bf16)
            nc.gpsimd.dma_start(perm[:, :], perm_e[:, :])
            ident = cpool.tile([128, 128], bf16)
            nc.gpsimd.dma_start(ident[:, :], ident_e[:, :])
            cos2 = cpool.tile([128, T], bf16)   # loaded per-chunk, streamed
            sin2 = cpool.tile([128, T], bf16)
            woutT = cpool.tile([128, 8 * HID], bf16)
            # ones row for the normalize broadcast: ones1^T @ rec (K=1 matmul)
            # replicates partition 0 of rec into 64 output partitions; the
            # multiply by 1.0 is exact so only rec's bf16 rounding enters
            ones1 = cpool.tile([1, DH], bf16)
            nc.vector.memset(ones1[0:1, :], 1.0)
            wtile = cpool.tile([128, 512], bf16)
            nc.vector.memset(wtile[:, :], 0.0)

            def emit_warm(n):
                # dependency-free matmuls: keep the PE HAM-busy (2.4 GHz)
                # through DMA waits / the final collective's network time
                for w in range(n):
                    wps = pspool.tile([128, 512], f32, tag="mm", bufs=2)
                    nc.tensor.matmul(wps[:, :], wtile[:, 0:128], wtile[:, :],
                                     start=True, stop=True)

            # ---- persistent working tensors ----
            q_sb = wpool.tile([128, T], bf16)      # raw q (rope intermediate)
            k_sb = wpool.tile([128, T], bf16)      # becomes roped k
            qzA = wpool.tile([128, T], bf16)       # roped qA rows 0-63, 0 below
            qzB = wpool.tile([128, T], bf16)       # roped qB rows 64-127, 0 above
            vT_sb = wpool.tile([128, T], bf16)     # v transposed [e, t]
            vexA = wpool.tile([128, 32 * 128], bf16)  # head-A v table per slot
            vexB = wpool.tile([128, 32 * 128], bf16)
            ovT = wpool.tile([128, T], bf16)       # attention out ^T
            PW = (TSB, TSB // 2, TSB // 2)     # per-core tokens per A2A piece
            PR = (0, TSB, TSB + TSB // 2)      # out_e row base per piece
            gTb = [wpool.tile([128, NCORES * PW[p]], bf16, name=f"gTb{p}")
                   for p in range(3)]              # post-A2A gathered [e, t]

            nc.vector.memset(qzA[DH:128, :], 0.0)
            nc.vector.memset(qzB[0:DH, :], 0.0)
            vexA3 = vexA.rearrange("p (s c) -> p s c", c=128)
            vexB3 = vexB.rearrange("p (s c) -> p s c", c=128)
            nc.vector.memset(vexA3[:, :, DH:128], 0.0)
            nc.vector.memset(vexB3[:, :, DH:128], 0.0)
            nc.vector.memset(vexA3[:, :, DH:DH + 1], 1.0)
            nc.vector.memset(vexB3[:, :, DH:DH + 1], 1.0)

            # ---------------- QKV + RoPE + v-transpose for one chunk ----------------
            def emit_qkv_chunk(c):
                xs = []
                for kt in range(8):
                    xt = spool.tile([128, CH], bf16, tag="xs", bufs=12)
                    nc.gpsimd.dma_start(
                        xt[:, :], xT_e[kt * 128:(kt + 1) * 128, c * CH:(c + 1) * CH]
                    )
                    xs.append(xt)
                sl = slice(c * CH, (c + 1) * CH)
                nc.gpsimd.dma_start(cos2[:, sl], cos2_e[:, sl])
                nc.gpsimd.dma_start(sin2[:, sl], sin2_e[:, sl])
                for which, dest in ((0, q_sb), (1, k_sb), (2, vT_sb)):
                    ps = pspool.tile([128, CH], f32, tag="mm", bufs=2)
                    for kt in range(8):
                        nc.tensor.matmul(
                            ps[:, :],
                            wqkvT[:, kt * 3 * EPC + which * EPC:
                                  kt * 3 * EPC + (which + 1) * EPC],
                            xs[kt][:, :],
                            start=(kt == 0),
                            stop=(kt == 7),
                        )
                    nc.vector.tensor_copy(dest[:, c * CH:(c + 1) * CH], ps[:, :])

                # RoPE on q and k: t = P@x * sin2 ; rot = x*cos2 + t
                for srd, dests in ((q_sb, ((qzA, 0, DH), (qzB, DH, 128))),
                                   (k_sb, ((k_sb, 0, 128),))):
                    pps = pspool.tile([128, CH], f32, tag="mm", bufs=2)
                    nc.tensor.matmul(
                        pps[:, :], perm[:, :], srd[:, sl],
                        start=True, stop=True,
                    )
                    tmp = spool.tile([128, CH], bf16, tag="ropetmp", bufs=2)
                    nc.vector.tensor_mul(tmp[:, :], pps[:, :], sin2[:, sl])
                    nc.vector.tensor_mul(srd[:, sl], srd[:, sl], cos2[:, sl])
                    for dst, p0, p1 in dests:
                        nc.vector.tensor_add(
                            dst[p0:p1, sl], srd[p0:p1, sl], tmp[p0:p1, :]
                        )

                # transpose v chunk into per-head 128-wide v tables
                for tt in range(CH // 128):
                    slot = c * (CH // 128) + tt
                    tp = pspool.tile([128, 128], bf16, tag="sc", bufs=2)
                    nc.tensor.transpose(
                        tp[:, :],
                        vT_sb[:, c * CH + tt * 128:c * CH + (tt + 1) * 128],
                        ident[:, :],
                    )
                    nc.vector.tensor_copy(vexA3[:, slot, 0:DH], tp[:, 0:DH])
                    nc.vector.tensor_copy(vexB3[:, slot, 0:DH], tp[:, DH:2 * DH])

            # ================= attention machinery =================
            NKT = N // KT                      # 16 key tiles per chunk
            vtabs = (vexA, vexB)

            def emit_pv_pair(st, pair):
                (b, qc, opsAB, expT) = st
                for h in range(HPC):
                    for kt in (2 * pair, 2 * pair + 1):
                        slot = b * (N // 128) + kt
                        nc.tensor.matmul(
                            opsAB[h][:, :],
                            vtabs[h][:, slot * 128:(slot + 1) * 128],
                            expT[:, kt * 1024 + h * QC:kt * 1024 + (h + 1) * QC],
                            start=(kt == 0),
                            stop=(kt == NKT - 1),
                        )

            def emit_normalize(st):
                (b, qc, opsAB, expT) = st
                q0 = b * N + qc * QC
                for h in range(HPC):
                    hr = h * DH
                    den = spool.tile([1, QC], f32, tag="den", bufs=2)
                    nc.vector.tensor_copy(den[0:1, :], opsAB[h][DH:DH + 1, :])
                    rec = spool.tile([1, QC], f32, tag="rec", bufs=2)
                    nc.vector.reciprocal_approx_fast(rec[0:1, :], den[0:1, :])
                    recb = spool.tile([1, QC], bf16, tag="recb", bufs=2)
                    nc.vector.tensor_copy(recb[0:1, :], rec[0:1, :])
                    bcs = pspool.tile([64, QC], f32, tag="sc", bufs=2)
                    nc.tensor.matmul(bcs[:, :], ones1[0:1, :], recb[0:1, :],
                                     start=True, stop=True)
                    # DVE may read only one PSUM operand; bounce bcs to SBUF
                    bcss = spool.tile([64, QC], f32, tag="bcss", bufs=2)
                    nc.vector.tensor_copy(bcss[:, :], bcs[:, :])
                    nc.vector.tensor_mul(
                        ovT[hr:hr + DH, q0:q0 + QC],
                        opsAB[h][0:DH, :], bcss[0:DH, :]
                    )

            def emit_stage(b, qc):
                # ship this finished chunk's blocks into its piece's A2A input
                if b == 0:
                    p, W, nb, jb = 0, TSB, 2, 2 * qc
                else:
                    p, W, nb, jb = 1 + qc // 2, TSB // 2, 4, 4 * (qc % 2)
                for i in range(nb):
                    t0 = b * N + qc * QC + i * W
                    nc.gpsimd.dma_start(
                        a2a_in[p][(jb + i) * 128:(jb + i + 1) * 128, :],
                        ovT[:, t0:t0 + W],
                    )

            def emit_outproj(p):
                W = PW[p]
                for m2 in range(W // 128):
                    for nn2 in range(HID // 512):
                        odps = pspool.tile([128, 512], f32, tag="mm", bufs=2)
                        for et in range(8):
                            nc.tensor.matmul(
                                odps[:, :],
                                gTb[p][:, et * W + m2 * 128:
                                       et * W + (m2 + 1) * 128],
                                woutT[:, et * HID + nn2 * 512:
                                      et * HID + (nn2 + 1) * 512],
                                start=(et == 0),
                                stop=(et == 7),
                            )
                        osb = spool.tile([128, 512], f32, tag="osb", bufs=2)
                        nc.vector.tensor_copy(osb[:, :], odps[:, :])
                        # sync queue: keeps gpsimd free for staging/doorbells
                        nc.sync.dma_start(
                            out_e[PR[p] + m2 * 128:PR[p] + (m2 + 1) * 128,
                                  nn2 * 512:(nn2 + 1) * 512],
                            osb[:, :],
                        )

            a2a_in = [dpool.tile([NCORES * 128, PW[p]], bf16, name=f"a2aI{p}")
                      for p in range(3)]
            a2a_out = [dpool.tile([NCORES * 128, PW[p]], bf16, name=f"a2aO{p}")
                       for p in range(3)]

            def emit_a2a(p):
                nc.gpsimd.collective_compute(
                    "AllToAll",
                    mybir.AluOpType.bypass,
                    ins=[a2a_in[p].opt()],
                    outs=[a2a_out[p].opt()],
                    replica_groups=[list(range(NCORES))],
                )

            def emit_gather(p):
                # one strided DMA: a2a_out [8(e) x 128(p), W(t)] ->
                # gTb [128(p), 8(e) x W(t)]; blocks on the A2A-done
                # semaphore, so it lives on the otherwise-idle sync queue
                src = a2a_out[p].rearrange("(e p) t -> p e t", p=128)
                dst = gTb[p].rearrange("p (e t) -> p e t", e=NCORES)
                nc.sync.dma_start(dst[:, :, :], src[:, :, :])

            # ================= emission =================
            emit_warm(12)          # cover the initial DMA wait
            # phase 1: batch-0 QKV only
            for c in range(4):
                emit_qkv_chunk(c)

            for kt in range(8):
                nc.gpsimd.dma_start(
                    woutT[:, kt * HID:(kt + 1) * HID],
                    woutT_e[kt * 128:(kt + 1) * 128, :],
                )

            qzs = (qzA, qzB)
            pending = None
            for b in range(B):
                for qc in range(N // QC):
                    q0 = b * N + qc * QC
                    expT = spool.tile([128, NKT * 2 * QC], bf16, name="expT",
                                      tag="expT", bufs=2)
                    for pair in range(NKT // 2):
                        for half in range(2):
                            kt = 2 * pair + half
                            k0 = b * N + kt * KT
                            sps = pspool.tile([128, 2 * QC], f32, tag="sc",
                                              bufs=2)
                            for h in range(HPC):
                                nc.tensor.matmul(
                                    sps[:, h * QC:(h + 1) * QC],
                                    k_sb[:, k0:k0 + KT],
                                    qzs[h][:, q0:q0 + QC],
                                    start=True, stop=True,
                                )
                            nc.scalar.activation(
                                expT[:, kt * 1024:(kt + 1) * 1024],
                                sps[:, :],
                                mybir.ActivationFunctionType.Exp,
                                scale=DH ** -0.5,
                            )
                        if pending is not None:
                            emit_pv_pair(pending, pair)
                            if pair == NKT // 2 - 1:
                                emit_normalize(pending)
                                pb, pqc = pending[0], pending[1]
                                emit_stage(pb, pqc)
                                if (pb, pqc) == (0, 3):
                                    emit_a2a(0)
                                elif (pb, pqc) == (1, 1):
                                    emit_a2a(1)
                    if b == 0:
                        # fuse batch-1 QKV into batch-0's (ACT-bound)
                        # attention chunks
                        emit_qkv_chunk(4 + qc)
                    if b == 1 and qc == 3:
                        # A2A#0 finished during qc1/qc2; emitting the gather +
                        # outproj here (not earlier) keeps the tile scheduler
                        # from head-of-line-blocking the PE on the A2A
                        emit_gather(0)
                        emit_outproj(0)
                    opsAB = (pspool.tile([128, QC], f32, name="opsA",
                                         tag="aux", bufs=2),
                             pspool.tile([128, QC], f32, name="opsB",
                                         tag="aux", bufs=2))
                    pending = (b, qc, opsAB, expT)
            for pair in range(NKT // 2):
                emit_pv_pair(pending, pair)
            emit_normalize(pending)
            emit_stage(1, 3)
            emit_a2a(2)
            emit_gather(1)
            emit_outproj(1)
            emit_warm(70)          # span A2A-Y's ~19us network time warm
            emit_gather(2)
            emit_outproj(2)

    nc.finalize()
    return nc


def _host_inputs(x, rope, Wqkv, Wout):
    """Build the 8 per-core input maps with host-side layout prep."""
    xf = np.ascontiguousarray(x.reshape(T, HID).T).astype(_bf16)        # [1024, 4096]
    woutT = np.ascontiguousarray(Wout.T).astype(_bf16)                  # [1024, 1024]

    rf = rope.reshape(T, DH)                                            # [4096, 64]
    cosE = np.repeat(rf[:, 0::2], 2, axis=1).T                          # [64, 4096]
    sinE = np.repeat(rf[:, 1::2], 2, axis=1).T
    sgn = np.where(np.arange(DH) % 2 == 0, -1.0, 1.0)[:, None]
    sinS = (sinE * sgn)
    cos2 = np.ascontiguousarray(np.concatenate([cosE, cosE], 0)).astype(_bf16)
    sin2 = np.ascontiguousarray(np.concatenate([sinS, sinS], 0)).astype(_bf16)

    pm = np.zeros((128, 128), np.float32)
    for d in range(128):
        pm[d ^ 1, d] = 1.0       # partner[d] = q[d^1]; lhsT = S (symmetric)
    perm = pm.astype(_bf16)
    ident = np.eye(128, dtype=np.float32).astype(_bf16)

    w3 = Wqkv.reshape(3, H, DH, HID)
    in_maps = []
    for c in range(NCORES):
        blocks = []
        for which in range(3):
            for hl in range(HPC):
                blocks.append(w3[which, 2 * c + hl])                    # [64, 1024]
        wq = np.concatenate(blocks, 0)                                  # [384, 1024]
        wqkvT = np.ascontiguousarray(wq.T).astype(_bf16)                # [1024, 384]
        in_maps.append({
            "xT": xf, "wqkvT": wqkvT, "woutT": woutT,
            "cos2": cos2, "sin2": sin2, "perm": perm, "ident": ident,
        })
    return in_maps


_CACHE = {}


def kernel(x, rope, Wqkv, Wout):
    from concourse.bass_utils import run_bass_kernel_spmd

    if "nc" not in _CACHE:
        _CACHE["nc"] = _build_graph()
    nc = _CACHE["nc"]
    in_maps = _host_inputs(np.asarray(x, np.float32), np.asarray(rope, np.float32),
                           np.asarray(Wqkv, np.float32), np.asarray(Wout, np.float32))
    res = run_bass_kernel_spmd(nc, in_maps, core_ids=list(range(NCORES)))
    parts = [np.asarray(res.results[i]["out"], np.float32) for i in range(NCORES)]
    full = np.empty((T, HID), np.float32)
    HW = TSB // 2
    for j in range(NCORES):
        full[j * TSB:(j + 1) * TSB] = parts[j][:TSB]
        full[N + j * HW:N + (j + 1) * HW] = parts[j][TSB:TSB + HW]
        full[N + N // 2 + j * HW:N + N // 2 + (j + 1) * HW] = parts[j][TSB + HW:]
    return full.reshape(B, N, HID)
